# revision 27
# baseline (speedup 1.0000x reference)
"""Trainium2 Bass kernel for nn_Alignment_vector (cross-attention alignment).

Data-parallel over batch across 8 NeuronCores (4 batches each). All three
matmuls run in bf16 on the TensorEngine; elementwise/normalization work is
fp32 on DVE/ACT.

Transposed operands (the attention contraction is over the contiguous `d`
axis): context goes through a bf16 DRAM staging copy + hardware x-bar
transpose DMA; qm = query*matrix and sim are transposed on the TensorEngine
(128x128 tiles against an identity) since they are already on chip.

Batches are software-pipelined with a skew of one so each in-order
sequencer issues batch b+1's loads before batch b's compute.

Math note: the softmax denominator cancels inside the following
l2_normalize, so softmax is computed as a bare exp().
"""

import numpy as np

import concourse.bacc as bacc
import concourse.tile as tile
import concourse.mybir as mybir
from concourse.masks import make_identity
from concourse.bass_utils import run_bass_kernel_spmd

f32 = mybir.dt.float32
bf16 = mybir.dt.bfloat16
AF = mybir.ActivationFunctionType
ALU = mybir.AluOpType

B, NCORES = 32, 8
BPC = B // NCORES            # batches per core
LQ, LS, D, S = 512, 1024, 1024, 256
NQ, NS, ND = LQ // 128, LS // 128, D // 128   # 4, 8, 8
EPS = 1e-8

LAST_EXEC_TIME_NS = None


def _build(smooth: float):
    nc = bacc.Bacc("TRN2", target_bir_lowering=False, debug=False)

    q_d = nc.dram_tensor("query", (BPC, LQ, D), f32, kind="ExternalInput").ap()
    c_d = nc.dram_tensor("context", (BPC, LS, D), f32, kind="ExternalInput").ap()
    m_d = nc.dram_tensor("matrix", (BPC, LQ, D), f32, kind="ExternalInput").ap()
    W_d = nc.dram_tensor("W", (S, D), f32, kind="ExternalInput").ap()
    bias_d = nc.dram_tensor("b", (S,), f32, kind="ExternalInput").ap()
    out_d = nc.dram_tensor("out", (BPC, LQ, S), f32, kind="ExternalOutput").ap()

    # bf16 staging areas for the context x-bar transpose (per-batch tensors
    # so Tile's whole-tensor DRAM dep tracking doesn't serialize batches)
    cx_s = [nc.dram_tensor(f"cx_s{i}", (LS, D), bf16, kind="Internal").ap()
            for i in range(BPC)]
    W_s = nc.dram_tensor("W_s", (S, D), bf16, kind="Internal").ap()

    with tile.TileContext(nc) as tc:
        from contextlib import ExitStack
        with ExitStack() as ctx:
            p = lambda *a, **k: ctx.enter_context(tc.tile_pool(*a, **k))
            qf_pool = p(name="qf", bufs=2)
            mf_pool = p(name="mf", bufs=1)
            qm_pool = p(name="qm", bufs=2)
            cx_pool = p(name="cx", bufs=2)
            cT_pool = p(name="cT", bufs=2)
            qT_pool = p(name="qT", bufs=1)
            al_pool = p(name="al", bufs=1)
            ee_pool = p(name="ee", bufs=1)
            sim_pool = p(name="sim", bufs=2)
            simT_pool = p(name="simT", bufs=1)
            wrk_pool = p(name="wrk", bufs=3)
            t2_pool = p(name="t2", bufs=1)
            sm_pool = p(name="sm", bufs=2)
            out_pool = p(name="outp", bufs=1)
            const_pool = p(name="const", bufs=1)
            psA_pool = p(name="psA", bufs=2, space="PSUM")
            psW_pool = p(name="psW", bufs=3, space="PSUM")
            psO_pool = p(name="psO", bufs=1, space="PSUM")
            psT_pool = p(name="psT", bufs=2, space="PSUM")

            # ---- once-per-core constants (issued after batch 0's loads
            # so they don't hog the DMA path during pipeline fill; WT/bB are
            # first needed by stage_y(0), much later) ----
            ident = const_pool.tile([128, 128], bf16)
            WT = const_pool.tile([128, ND, S], bf16)
            bB = const_pool.tile([128, S], f32)

            def const_setup():
                make_identity(nc, ident[:])
                Wsb = qm_pool.tile([128, S // 128, D], bf16, tag="qm")
                nc.gpsimd.dma_start(
                    Wsb[:], W_d.rearrange("(t p) d -> p t d", p=128))
                nc.scalar.dma_start(
                    W_s.rearrange("(t p) d -> p t d", p=128), Wsb[:])
                nc.sync.dma_start(WT[:], W_s, transpose=True)
                ones_c = const_pool.tile([1, 128], f32)
                nc.vector.memset(ones_c[:], 1.0)
                b_sb = const_pool.tile([1, S], f32)
                nc.gpsimd.dma_start(b_sb[:],
                                    bias_d.rearrange("(o s) -> o s", o=1))
                ps_b = psO_pool.tile([128, S], f32, tag="psO")
                nc.tensor.matmul(ps_b[:], lhsT=ones_c[:], rhs=b_sb[:],
                                 start=True, stop=True)
                nc.vector.tensor_copy(bB[:], ps_b[:])

            def pe_transpose(src, srclice, dst, dslice_of):
                """Transpose NQ*ND 128x128 tiles of src into dst via the PE.
                src[:, t, 128k:128(k+1)] -> dst[:, k, 128t:128(t+1)].
                Packs ND transposes per PSUM bank, one DVE copy per pack."""
                nt = src.shape[1]
                for t in range(nt):
                    pst = psT_pool.tile([128, ND, 128], bf16, tag="psT")
                    for k in range(ND):
                        nc.tensor.transpose(
                            pst[:, k, :], src[:, t, 128 * k:128 * (k + 1)],
                            ident[:])
                    nc.vector.tensor_copy(
                        dst[:, :, 128 * t:128 * (t + 1)], pst[:])

            def stage_ab(bi):
                """Loads, qm product, context staging store + transposes."""
                qf = qf_pool.tile([128, NQ, D], f32)
                nc.gpsimd.dma_start(
                    qf[:], q_d[bi].rearrange("(t p) d -> p t d", p=128))
                mf = mf_pool.tile([128, NQ, D], bf16)
                nc.gpsimd.dma_start(
                    mf[:], m_d[bi].rearrange("(t p) d -> p t d", p=128))
                qm = qm_pool.tile([128, NQ, D], bf16, tag="qm")
                nc.vector.tensor_tensor(out=qm[:], in0=qf[:], in1=mf[:],
                                        op=ALU.mult)
                cx = cx_pool.tile([128, NS, D], bf16)
                nc.gpsimd.dma_start(
                    cx[:], c_d[bi].rearrange("(t p) d -> p t d", p=128))
                # context halves: store + x-bar as soon as each half loads
                cT = cT_pool.tile([128, ND, LS], bf16)
                for h in range(2):
                    rows = slice(512 * h, 512 * (h + 1))
                    nc.scalar.dma_start(
                        cx_s[bi][rows].rearrange("(t p) d -> p t d", p=128),
                        cx[:, 4 * h:4 * (h + 1), :])
                    nc.sync.dma_start(cT[:, :, rows], cx_s[bi][rows],
                                      transpose=True)
                return dict(qf=qf, cx=cx, qm=qm, cT=cT)

            def stage_x1(bi, t):
                qf, cx, qm, cT = t["qf"], t["cx"], t["qm"], t["cT"]

                # qT = qm.T via PE (keeps PE warm while cT transposes arrive)
                qT = qT_pool.tile([128, ND, LQ], bf16)
                pe_transpose(qm, None, qT, None)

                # ---- C: attn = lrelu(context @ qm.T), row-normalize,
                #      E = exp(smooth * attn / ||row||) ----
                AL = al_pool.tile([128, NS, LQ], bf16)
                ss = sm_pool.tile([128, NS], f32, tag="ss")
                for m in range(NS):
                    psA = psA_pool.tile([128, LQ], f32)
                    for k in range(ND):
                        nc.tensor.matmul(
                            psA[:], lhsT=cT[:, k, 128 * m:128 * (m + 1)],
                            rhs=qT[:, k, :],
                            start=(k == 0), stop=(k == ND - 1))
                    t01 = wrk_pool.tile([128, LQ], bf16, tag="t01")
                    if m % 2 == 0:
                        nc.scalar.mul(t01[:], psA[:], 0.1)
                    else:
                        nc.vector.tensor_scalar_mul(t01[:], psA[:], 0.1)
                    nc.vector.tensor_tensor(out=AL[:, m, :], in0=psA[:],
                                            in1=t01[:], op=ALU.max)
                    sqd = wrk_pool.tile([128, LQ], bf16, tag="sqd")
                    nc.scalar.activation(sqd[:], AL[:, m, :], AF.Square,
                                         accum_out=ss[:, m:m + 1])
                return dict(t=t, AL=AL, ss=ss)

            def stage_x2(bi, s):
                t, AL, ss = s["t"], s["AL"], s["ss"]
                qf, cx, qm, cT = t["qf"], t["cx"], t["qm"], t["cT"]
                rs = sm_pool.tile([128, NS], f32, tag="rs")
                nc.scalar.sqrt(rs[:], ss[:])
                nc.vector.reciprocal(rs[:], rs[:])
                nc.vector.tensor_scalar_mul(rs[:], rs[:], float(smooth))
                E = ee_pool.tile([128, NS, LQ], bf16)
                for m in range(NS):
                    nc.scalar.activation(E[:, m, :], AL[:, m, :], AF.Exp,
                                         scale=rs[:, m:m + 1])

                # ---- D: wc = E.T @ context, l2-normalize rows,
                #      sim = (query - wcn)^2 ----
                sim = sim_pool.tile([128, NQ, D], bf16)
                for mq in range(NQ):
                    ssw = sm_pool.tile([128, 2], f32, tag="ssw")
                    ps_h = []
                    for n in range(2):
                        sl = slice(512 * n, 512 * (n + 1))
                        psW = psW_pool.tile([128, 512], f32, tag="psW")
                        for k in range(NS):
                            nc.tensor.matmul(
                                psW[:],
                                lhsT=E[:, k, 128 * mq:128 * (mq + 1)],
                                rhs=cx[:, k, sl],
                                start=(k == 0), stop=(k == NS - 1))
                        sq0 = wrk_pool.tile([128, 512], bf16, tag="sqd")
                        nc.scalar.activation(sq0[:], psW[:], AF.Square,
                                             accum_out=ssw[:, n:n + 1])
                        ps_h.append(psW)
                    g = sm_pool.tile([128, 1], f32, tag="g")
                    nc.vector.tensor_tensor(out=g[:], in0=ssw[:, 0:1],
                                            in1=ssw[:, 1:2], op=ALU.add)
                    nc.scalar.sqrt(g[:], g[:])
                    nc.vector.reciprocal(g[:], g[:])
                    for n in range(2):
                        sl = slice(512 * n, 512 * (n + 1))
                        wn = wrk_pool.tile([128, 512], bf16, tag="wn")
                        nc.vector.tensor_scalar_mul(wn[:], ps_h[n][:], g[:])
                        tt = wrk_pool.tile([128, 512], f32, tag="tt")
                        nc.vector.tensor_tensor(out=tt[:], in0=qf[:, mq, sl],
                                                in1=wn[:], op=ALU.subtract)
                        nc.scalar.activation(sim[:, mq, sl], tt[:], AF.Square)

                return sim

            def stage_y(bi, sim):
                # simT = sim.T via PE
                simT = simT_pool.tile([128, ND, LQ], bf16)
                pe_transpose(sim, None, simT, None)
                # ---- E: out = l2norm(sim @ W.T + b) ----
                outT = out_pool.tile([128, NQ, S], f32)
                ss3 = sm_pool.tile([128, NQ], f32, tag="ss3")
                t2s = []
                for mq in range(NQ):
                    psO = psO_pool.tile([128, S], f32, tag="psO")
                    for k in range(ND):
                        nc.tensor.matmul(
                            psO[:], lhsT=simT[:, k, 128 * mq:128 * (mq + 1)],
                            rhs=WT[:, k, :],
                            start=(k == 0), stop=(k == ND - 1))
                    t2 = t2_pool.tile([128, S], f32, tag=f"t2_{mq}")
                    nc.vector.tensor_tensor(out=t2[:], in0=psO[:], in1=bB[:],
                                            op=ALU.add)
                    sq3 = wrk_pool.tile([128, S], bf16, tag="sqd")
                    nc.scalar.activation(sq3[:], t2[:], AF.Square,
                                         accum_out=ss3[:, mq:mq + 1])
                    t2s.append(t2)
                nc.scalar.sqrt(ss3[:], ss3[:])
                nc.vector.tensor_scalar_add(ss3[:], ss3[:], EPS)
                nc.vector.reciprocal(ss3[:], ss3[:])
                for mq in range(NQ):
                    nc.vector.tensor_scalar_mul(outT[:, mq, :], t2s[mq][:],
                                                ss3[:, mq:mq + 1])
                nc.scalar.dma_start(
                    out_d[bi].rearrange("(t p) s -> p t s", p=128), outT[:])

            # software pipeline: AB skew 1; Y(b-1) issued between X1(b)
            # (MM1 block) and X2(b) (MM2 block) so its PE work fills batch
            # b's softmax-chain stall
            tiles = {0: stage_ab(0)}
            const_setup()
            sims = {}
            for bi in range(1, BPC):
                tiles[bi] = stage_ab(bi)
                st = stage_x1(bi - 1, tiles.pop(bi - 1))
                if bi - 2 >= 0:
                    stage_y(bi - 2, sims.pop(bi - 2))
                sims[bi - 1] = stage_x2(bi - 1, st)
            st = stage_x1(BPC - 1, tiles.pop(BPC - 1))
            if BPC - 2 >= 0:
                stage_y(BPC - 2, sims.pop(BPC - 2))
            sims[BPC - 1] = stage_x2(BPC - 1, st)
            for bi in sorted(sims):
                stage_y(bi, sims.pop(bi))

    nc.compile()
    return nc


_NC_CACHE: dict = {}


def kernel(query, context, matrix, W, b, smooth):
    global LAST_EXEC_TIME_NS
    sm = float(smooth)
    nc = _NC_CACHE.get(sm)
    if nc is None:
        nc = _build(sm)
        _NC_CACHE[sm] = nc

    query = np.ascontiguousarray(query, dtype=np.float32)
    context = np.ascontiguousarray(context, dtype=np.float32)
    matrix = np.ascontiguousarray(matrix, dtype=np.float32)
    W = np.ascontiguousarray(W, dtype=np.float32)
    b = np.ascontiguousarray(b, dtype=np.float32)

    in_maps = []
    for c in range(NCORES):
        sl = slice(c * BPC, (c + 1) * BPC)
        in_maps.append({
            "query": query[sl],
            "context": context[sl],
            "matrix": matrix[sl],
            "W": W,
            "b": b,
        })
    res = run_bass_kernel_spmd(nc, in_maps, core_ids=list(range(NCORES)))
    LAST_EXEC_TIME_NS = res.exec_time_ns
    out = np.concatenate([r["out"] for r in res.results], axis=0)
    return out
